# revision 13
# baseline (speedup 1.0000x reference)
"""Trainium2 Bass kernel for nn_MultiHeadedAttention (B=4, S=1024, D=1024, H=16).

Sharding: 8 cores = 4 batches x 2 head-halves (8 heads each). The reference's
row-major reshape after [B,H,S,d] means output row r = h*64 + s//16 depends
only on head h, so head sharding needs no collective: each core computes a
[512, 1024] row-block of its batch's output.

Per-core pipeline (all matmuls contract on the partition dim):
  QT/KT = WxT.T @ XxT          -> [j, s] layout (head dims on partitions)
  V     = XvT.T @ WvT          -> [s, j] natural layout, augmented with a
                                  ones column per head (row 64 of PV psum
                                  then accumulates the softmax denominator)
  scoresT[k, q] = KT_h.T @ QT_h  (q in s16-major order so PV output lands in
                                  the layout the final reshape needs)
  wT = exp(0.125 * scoresT)      (mask is a no-op unless mask@mask.T has
                                  zeros; host checks and enables a penalty-add
                                  fallback path in that case)
  xT'[dd|sum, q] = V_aug.T @ wT  (accumulated over k tiles)
  lhsT = xT'[0:64] * (1/sum)     (DVE copy into x_block.T layout, 2 heads
                                  side by side)
  out  = lhsT.T @ WoT            -> [128 rows, 1024] per head pair, DMA'd out.
"""

import numpy as np

import concourse.bass as bass
import concourse.bacc as bacc
import concourse.tile as tile
from concourse import mybir
from concourse.bass_utils import run_bass_kernel_spmd

F32 = mybir.dt.float32

B, S, D, H = 4, 1024, 1024, 16
d_head = D // H  # 64
HPC = 8          # heads per core
JC = HPC * d_head  # 512 columns of W per core

_cached = {}


def build_program(use_mask: bool):
    nc = bacc.Bacc(None, target_bir_lowering=False, debug=False)

    xqT = nc.dram_tensor("xqT", [D, S], F32, kind="ExternalInput").ap()
    xkT = nc.dram_tensor("xkT", [D, S], F32, kind="ExternalInput").ap()
    xvT = nc.dram_tensor("xvT", [D, S], F32, kind="ExternalInput").ap()
    wqT = nc.dram_tensor("wqT", [D, JC], F32, kind="ExternalInput").ap()
    wkT = nc.dram_tensor("wkT", [D, JC], F32, kind="ExternalInput").ap()
    wvT = nc.dram_tensor("wvT", [D, JC], F32, kind="ExternalInput").ap()
    bq_col = nc.dram_tensor("bq_col", [128, 4], F32, kind="ExternalInput").ap()
    bk_col = nc.dram_tensor("bk_col", [128, 4], F32, kind="ExternalInput").ap()
    bv_bc = nc.dram_tensor("bv_bc", [128, JC], F32, kind="ExternalInput").ap()
    woT = nc.dram_tensor("woT", [D, D], F32, kind="ExternalInput").ap()
    if use_mask:
        pen = nc.dram_tensor("pen", [S, S], F32, kind="ExternalInput").ap()
    out = nc.dram_tensor("out", [JC, D], F32, kind="ExternalOutput").ap()

    with tile.TileContext(nc) as tc:
        with (
            tc.tile_pool(name="big", bufs=16) as big,        # [128,1024] tiles
            tc.tile_pool(name="wp", bufs=16) as wp,          # [128,512] tiles
            tc.tile_pool(name="qt", bufs=4) as qt_p,
            tc.tile_pool(name="kt", bufs=4) as kt_p,
            tc.tile_pool(name="va", bufs=8) as va_p,
            tc.tile_pool(name="wT", bufs=4) as wT_p,
            tc.tile_pool(name="lh", bufs=2) as lh_p,
            tc.tile_pool(name="outp", bufs=3) as outp,
            tc.tile_pool(name="small", bufs=6) as smallp,
            tc.tile_pool(name="psA", bufs=2, space="PSUM") as psA,
            tc.tile_pool(name="psB", bufs=4, space="PSUM") as psB,
        ):
            # bias tiles
            bq_sb = smallp.tile([128, 4], F32, tag="bias", bufs=2)
            nc.sync.dma_start(bq_sb[:], bq_col[:])
            bk_sb = smallp.tile([128, 4], F32, tag="bias", bufs=2)
            nc.sync.dma_start(bk_sb[:], bk_col[:])
            bv_sb = smallp.tile([128, JC], F32, tag="biasr", bufs=1)
            nc.sync.dma_start(bv_sb[:], bv_bc[:])

            # ---- Q / K projections -> QT/KT [j, s] ----
            def proj_qk(xT, wT_d, bias_sb, dst_pool):
                xt = []
                for dt in range(8):
                    t = big.tile([128, S], F32, tag="x")
                    nc.sync.dma_start(t[:], xT[dt * 128:(dt + 1) * 128, :])
                    xt.append(t)
                wt = []
                for dt in range(8):
                    t = wp.tile([128, JC], F32, tag="w")
                    nc.sync.dma_start(t[:], wT_d[dt * 128:(dt + 1) * 128, :])
                    wt.append(t)
                dst = [dst_pool.tile([128, S], F32, tag="dst", name="dst") for _ in range(4)]
                for jt in range(4):
                    for st in range(2):
                        ps = psB.tile([128, 512], F32, tag="ps1")
                        for dt in range(8):
                            nc.tensor.matmul(
                                ps[:],
                                lhsT=wt[dt][:, jt * 128:(jt + 1) * 128],
                                rhs=xt[dt][:, st * 512:(st + 1) * 512],
                                start=(dt == 0),
                                stop=(dt == 7),
                            )
                        nc.vector.tensor_scalar_add(
                            dst[jt][:, st * 512:(st + 1) * 512], ps[:],
                            bias_sb[:, jt:jt + 1],
                        )
                return dst

            QT = proj_qk(xqT, wqT, bq_sb, qt_p)
            KT = proj_qk(xkT, wkT, bk_sb, kt_p)

            # ---- V projection -> V_aug [s, 8*65] (65th col per head = 1.0) ----
            xvt = []
            for dt in range(8):
                t = big.tile([128, S], F32, tag="x")
                nc.sync.dma_start(t[:], xvT[dt * 128:(dt + 1) * 128, :])
                xvt.append(t)
            wvt = []
            for dt in range(8):
                t = wp.tile([128, JC], F32, tag="w")
                nc.sync.dma_start(t[:], wvT[dt * 128:(dt + 1) * 128, :])
                wvt.append(t)
            VA = []
            for st in range(8):
                ps = psB.tile([128, 512], F32, tag="ps1")
                for dt in range(8):
                    nc.tensor.matmul(
                        ps[:],
                        lhsT=xvt[dt][:, st * 128:(st + 1) * 128],
                        rhs=wvt[dt][:],
                        start=(dt == 0),
                        stop=(dt == 7),
                    )
                va = va_p.tile([128, 8 * 65], F32)
                nc.vector.memset(va[:], 1.0)
                nc.vector.tensor_tensor(
                    va[:].rearrange("p (h c) -> p h c", h=8)[:, :, 0:64],
                    ps[:].rearrange("p (h c) -> p h c", h=8),
                    bv_sb[:].rearrange("p (h c) -> p h c", h=8),
                    op=mybir.AluOpType.add,
                )
                VA.append(va)

            # woT tiles (reuse big pool slots released by x tiles)
            wo_t = []
            for ct in range(8):
                t = big.tile([128, D], F32, tag="x")
                nc.sync.dma_start(t[:], woT[ct * 128:(ct + 1) * 128, :])
                wo_t.append(t)

            pen_t = []
            if use_mask:
                for kt in range(8):
                    t = big.tile([128, S], F32, tag="x")
                    nc.sync.dma_start(t[:], pen[kt * 128:(kt + 1) * 128, :])
                    pen_t.append(t)

            def QT_perm(hl, qch):
                # rhs [64, 512] with q in s16-major order:
                # col j reads s = q16*16 + s16, s16 = qch*8 + j//64, q16 = j%64
                tile_ = QT[hl // 2]
                po = (hl % 2) * 64
                ap = tile_[po:po + 64, :].rearrange("p (q s) -> p s q", s=16)
                return ap[:, qch * 8:(qch + 1) * 8, :]

            def KT_ap(hl, kt):
                tile_ = KT[hl // 2]
                po = (hl % 2) * 64
                return tile_[po:po + 64, kt * 128:(kt + 1) * 128]

            # ---- attention per head pair ----
            for p in range(4):
                hA, hB = 2 * p, 2 * p + 1
                pv = {}
                for kt in range(8):
                    scA = psA.tile([128, 1024], F32, tag="sc")
                    scB = psA.tile([128, 1024], F32, tag="sc")
                    # interleave A/B so row-groups 0-63 / 64-127 overlap on PE
                    for qch in range(2):
                        nc.tensor.matmul(
                            scA[:, qch * 512:(qch + 1) * 512],
                            lhsT=KT_ap(hA, kt), rhs=QT_perm(hA, qch),
                            start=True, stop=True,
                        )
                        nc.tensor.matmul(
                            scB[:, qch * 512:(qch + 1) * 512],
                            lhsT=KT_ap(hB, kt), rhs=QT_perm(hB, qch),
                            start=True, stop=True,
                        )
                    if use_mask:
                        # pen[k, q] with the same s16-major q permutation
                        pap = pen_t[kt][:].rearrange("p (q s) -> p s q", s=16)
                        for sc in (scA, scB):
                            nc.vector.tensor_tensor(
                                sc[:].rearrange("p (s q) -> p s q", s=16),
                                sc[:].rearrange("p (s q) -> p s q", s=16),
                                pap, op=mybir.AluOpType.add,
                            )
                    wA = wT_p.tile([128, 1024], F32, tag="wT")
                    wB = wT_p.tile([128, 1024], F32, tag="wT")
                    nc.scalar.activation(wA[:], scA[:],
                                         mybir.ActivationFunctionType.Exp,
                                         scale=0.125)
                    nc.scalar.activation(wB[:], scB[:],
                                         mybir.ActivationFunctionType.Exp,
                                         scale=0.125)
                    for i, (hl, wt_, qch) in enumerate(
                        [(hA, wA, 0), (hB, wB, 0), (hA, wA, 1), (hB, wB, 1)]
                    ):
                        if kt == 0:
                            pv[i] = psB.tile([65, 512], F32, tag="ps1", name="pv")
                        nc.tensor.matmul(
                            pv[i][:],
                            lhsT=VA[kt][:, hl * 65:hl * 65 + 65],
                            rhs=wt_[:, qch * 512:(qch + 1) * 512],
                            start=(kt == 0), stop=(kt == 7),
                        )

                # normalize + shuffle into final-projection lhsT layout
                lh = lh_p.tile([128, 1024], F32)
                for hloc, hl in enumerate((hA, hB)):
                    rc = smallp.tile([64, 1024], F32, tag="rc", bufs=2)
                    rcb = smallp.tile([64, 1024], F32, tag="rcb", bufs=2)
                    nc.vector.memset(rc[:], 1.0)  # keep bcast input finite
                    for qch in range(2):
                        i = hloc + 2 * qch
                        nc.vector.reciprocal(
                            rc[0:1, qch * 512:(qch + 1) * 512], pv[i][64:65, :])
                    nc.gpsimd.partition_broadcast(rcb[:], rc[:])
                    rcv = rcb[:].rearrange("p (s q) -> p s q", s=16)
                    for qch in range(2):
                        i = hloc + 2 * qch
                        src = pv[i][0:64, :].rearrange("p (s q) -> p s q", s=8)
                        for par, off in ((0, 0), (1, 64)):  # even/odd s16
                            # lh layout: [part, (ct 8)(head 2)(q16 64)] so the
                            # final matmul's lhsT tile ct is one contiguous
                            # 128-col block (walrus: stationary AP needs a
                            # single free dim)
                            dst = lh[off:off + 64, :].rearrange(
                                "p (c m) -> p c m", c=8
                            )[:, qch * 4:(qch + 1) * 4,
                              hloc * 64:(hloc + 1) * 64]
                            nc.vector.tensor_tensor(
                                dst,
                                src[:, par::2, :],
                                rcv[:, qch * 8 + par:qch * 8 + 8:2, :],
                                op=mybir.AluOpType.mult,
                            )

                # final projection: out rows p*128 .. p*128+128
                for ot in range(2):
                    fp = psB.tile([128, 512], F32, tag="ps1")
                    for ct in range(8):
                        nc.tensor.matmul(
                            fp[:],
                            lhsT=lh[:, ct * 128:(ct + 1) * 128],
                            rhs=wo_t[ct][:, ot * 512:(ot + 1) * 512],
                            start=(ct == 0), stop=(ct == 7),
                        )
                    ob = outp.tile([128, 512], F32)
                    nc.vector.tensor_copy(ob[:], fp[:])
                    nc.sync.dma_start(
                        out[p * 128:(p + 1) * 128, ot * 512:(ot + 1) * 512],
                        ob[:],
                    )

    nc.compile()
    return nc


def make_in_maps(query, key, value, mask, Wq, bq, Wk, bk, Wv, bv, Wo,
                 pen_b=None):
    woT = np.ascontiguousarray(Wo.T)
    maps = []
    for c in range(8):
        b, hf = c // 2, c % 2
        sl = slice(hf * JC, (hf + 1) * JC)
        m = {
            "xqT": np.ascontiguousarray(query[b].T),
            "xkT": np.ascontiguousarray(key[b].T),
            "xvT": np.ascontiguousarray(value[b].T),
            "wqT": np.ascontiguousarray(Wq[sl].T),
            "wkT": np.ascontiguousarray(Wk[sl].T),
            "wvT": np.ascontiguousarray(Wv[sl].T),
            "bq_col": np.ascontiguousarray(bq[sl].reshape(4, 128).T),
            "bk_col": np.ascontiguousarray(bk[sl].reshape(4, 128).T),
            "bv_bc": np.ascontiguousarray(
                np.broadcast_to(bv[sl].reshape(1, JC), (128, JC))),
            "woT": woT,
        }
        if pen_b is not None:
            m["pen"] = pen_b[b]
        maps.append(m)
    return maps


def kernel(query, key, value, mask, Wq, bq, Wk, bk, Wv, bv, Wo):
    query = np.asarray(query, np.float32)
    key = np.asarray(key, np.float32)
    value = np.asarray(value, np.float32)
    mask = np.asarray(mask, np.float32)

    m2d = mask[0]  # [B, S, 64]
    mm = np.stack([m2d[b] @ m2d[b].T for b in range(B)])  # [B, S, S]
    use_mask = bool((mm == 0).any())
    pen_b = None
    if use_mask:
        pen_b = np.where(mm == 0, np.float32(-1e9), np.float32(0.0))
        pen_b = np.ascontiguousarray(pen_b, np.float32)

    if use_mask not in _cached:
        _cached[use_mask] = build_program(use_mask)
    nc = _cached[use_mask]

    in_maps = make_in_maps(query, key, value, mask,
                           np.asarray(Wq, np.float32), np.asarray(bq, np.float32),
                           np.asarray(Wk, np.float32), np.asarray(bk, np.float32),
                           np.asarray(Wv, np.float32), np.asarray(bv, np.float32),
                           np.asarray(Wo, np.float32), pen_b)
    res = run_bass_kernel_spmd(nc, in_maps, list(range(8)))

    out = np.empty((B, S, D), np.float32)
    for c in range(8):
        b, hf = c // 2, c % 2
        out[b, hf * JC:(hf + 1) * JC, :] = res.results[c]["out"]
    return out
